# revision 1
# baseline (speedup 1.0000x reference)
"""NT-Xent contrastive loss (forward) on 8 TRN2 NeuronCores via Bass/Tile.

Math: with h = concat(h_i, h_j) [N=8192, D=256], sim = (h @ h.T) / 0.5,
loss = mean_r( logsumexp_j(sim[r, j], j != r) - pos_r ), where
pos_r = sim[r, partner(r)] = 2 * h_i[q] . h_j[q].  The loss separates:
loss = (sum_r lse_r - sum_r pos_r) / N, and sum_r pos_r = 4 * sum(h_i * h_j).

Sharding: core c owns rows [1024c, 1024c + 1024).  Each core receives the
full transposed h, column-rotated by its row offset, so one SPMD program
serves all 8 cores: the self-similarity diagonal and the positive-pair
columns land at core-invariant positions.

Per core: the PE builds each 128-row block of sim in PSUM (bf16 operands,
fp32 accumulate) as four 1536-column chunks + two 1024-column chunks; the
diagonal is masked by accumulating I.T @ (-1e9 shifted-diag) as an extra
matmul; the scalar engine applies exp(2x - M_row) in place with a fused
row-sum (accum_out) on the 1536-chunks while the vector engine evaluates a
Schraudolph bit-trick exp (+-4%% per term, unbiased on average) on the
1024-chunks; the DVE also computes the positive-pair partial dots.  Each
core emits a [128, 52] tile of partial sums; the host finishes with
log/sum in float64.  M is a runtime input (per-row); if a row's exp-sum
under/overflows fp32, the host retries with a shifted M for those rows.
"""

import numpy as np
import ml_dtypes

B = 4096
D = 256
N = 2 * B            # 8192 rows/cols of sim
NCORES = 8
RPC = N // NCORES    # 1024 rows per core
KCH = D // 128       # 2 contraction chunks of 128
NJ = 4               # column chunks per row-block
CHUNK = N // NJ      # 2048 columns per chunk
NRB = RPC // 128     # 8 row-blocks of 128 per core
M_DEFAULT = 161.0    # logsumexp shift; safe while rowmax(2*h@h.T) in [M-70, M+79]
MASK_NEG = -1.0e9

# Schraudolph fast-exp constants (exp(y) ~= bitcast_f32(round(A*y + B)));
# B calibrated so the phase-averaged, exp-weighted relative error is ~1e-5
# (per-term max +-4%).  The DVE evaluates this for 1 of 4 column chunks per
# row-block, offloading a quarter of the exp work from the scalar engine;
# negative overflow saturates to INT_MIN = -0.0f which sums as zero.
EXP_A = float(2 ** 23 / np.log(2.0))
EXP_B = 1064865216.0

TRACE = False        # set True (e.g. from test.py) to request an NTFF trace
LAST_RESULTS = None  # BassKernelResults of the last run (for profiling)

_cache = {}


def _build():
    """Build the SPMD Bass/Tile program once per process."""
    if "nc" in _cache:
        return _cache["nc"]

    import concourse.tile as tile
    import concourse.mybir as mybir
    from concourse import bacc

    f32 = mybir.dt.float32
    bf16 = mybir.dt.bfloat16
    u32 = mybir.dt.uint32

    nc = bacc.Bacc("TRN2", target_bir_lowering=False, num_devices=NCORES)
    ht_dram = nc.dram_tensor("ht", [KCH, 128, N], bf16, kind="ExternalInput").ap()
    # eye[0][0] = I [128,128]; mask[v] [128,512] holds -1e9 at [p, 128v+p].
    # I.T @ mask[v] accumulated into a sim-block 512-slice masks its diagonal.
    eye_dram = nc.dram_tensor("eye", [1, 128, 128], bf16, kind="ExternalInput").ap()
    maskr_dram = nc.dram_tensor("maskr", [128, 4, 512], bf16, kind="ExternalInput").ap()
    bias_dram = nc.dram_tensor("biasm", [128, NRB], f32, kind="ExternalInput").ap()
    bias2_dram = nc.dram_tensor("bias2", [128, NRB], f32, kind="ExternalInput").ap()
    out_dram = nc.dram_tensor("out", [128, 52], f32, kind="ExternalOutput").ap()

    with tile.TileContext(nc) as tc:
        with (
            tc.tile_pool(name="hpool", bufs=1) as hpool,
            tc.tile_pool(name="small", bufs=1) as small,
            tc.tile_pool(name="scratch", bufs=1) as scratch,
            tc.tile_pool(name="ipool", bufs=3) as ipool,
            tc.tile_pool(name="psumA", bufs=2, space="PSUM") as psumA,
            tc.tile_pool(name="psumB", bufs=1, space="PSUM") as psumB,
        ):
            # Small constants go on the gpsimd (SWDGE) queue so they land
            # while the sync queue streams the big h.T chunks.
            eye_pos = small.tile([128, 128], bf16)
            nc.gpsimd.dma_start(out=eye_pos, in_=eye_dram[0])
            maskr_sb = small.tile([128, 4, 512], bf16)
            nc.gpsimd.dma_start(out=maskr_sb, in_=maskr_dram)
            bias_sb = small.tile([128, NRB], f32)
            nc.gpsimd.dma_start(out=bias_sb, in_=bias_dram)
            bias2_sb = small.tile([128, NRB], f32)
            nc.gpsimd.dma_start(out=bias2_sb, in_=bias2_dram)

            # Warm the ACT exp table (~2.7us load) during the DMA prologue so
            # the first real exp doesn't pay for it.
            warm_sb = small.tile([128, 1], f32)
            nc.scalar.activation(
                out=warm_sb, in_=bias_sb[:, 0:1],
                func=mybir.ActivationFunctionType.Exp, bias=0.0, scale=0.0,
            )

            # Warm the PE's HAM clock gate (cold = 1.2GHz for the first
            # ~3.4us of activity) with dummy matmuls on a memset tile while
            # the h.T DMAs are still in flight.
            wsrc = small.tile([128, 128], bf16)
            nc.vector.memset(wsrc, 0.0)
            wps = psumA.tile([128, 1536], f32, name="psA")
            for w in range(32):
                nc.tensor.matmul(
                    wps[:, (w % 3) * 512:(w % 3) * 512 + 128],
                    lhsT=wsrc, rhs=wsrc,
                    start=True, stop=True,
                )

            # h.T in SBUF on the sync HWDGE queue, in the order compute
            # consumes it.  Each DMA carries BOTH contraction halves of a
            # column range (tile layout [128, 2, width]) so the pipeline
            # never waits on a second transfer for the same columns.
            col_ranges = [(0, 1024), (1024, 2048), (2048, 3584),
                          (3584, 5120), (5120, 6656), (6656, 8192)]
            ht_tiles = []
            for di, (c0, c1) in enumerate(col_ranges):
                t = hpool.tile([128, KCH, c1 - c0], bf16, name=f"ht_{c0}")
                nc.sync.dma_start(
                    out=t,
                    in_=ht_dram[:, :, c0:c1].rearrange("k p c -> p k c"),
                )
                ht_tiles.append(t)

            def rhs_slice(k, c0, w=512):
                """[128, w] slice of rotated h.T at global column c0."""
                for (r0, r1), t in zip(col_ranges, ht_tiles):
                    if r0 <= c0 < r1:
                        assert c0 + w <= r1
                        return t[:, k, c0 - r0:c0 - r0 + w]
                raise AssertionError(c0)

            def lhsT_slice(k, rb):
                """[128, 128] row-block weights (columns rb*128..+128)."""
                return ht_tiles[0][:, k, rb * 128:(rb + 1) * 128]

            res_sb = small.tile([128, 52], f32)

            # Per row-block: 4 ACT chunks of 1536 columns (two 3-bank PSUM
            # slots) + 2 DVE fast-exp chunks of 1024 columns (one 2-bank
            # slot) = exactly 8 PSUM banks, with enough slot slack that the
            # PE never waits on a consumer.
            def emit_posdot():
                # Positive-pair partial dots: rotated columns [0,1024) are
                # this core's rows, [4096,5120) their partners.  Emitted
                # mid-stream so the DVE does it in slack, not on the tail.
                for k in range(KCH):
                    pp = scratch.tile([128, RPC], f32, name=f"ppscratch_{k}")
                    nc.vector.tensor_mul(pp, ht_tiles[0][:, k, :], ht_tiles[3][:, k, 512:512 + RPC])
                    nc.vector.reduce_sum(
                        res_sb[:, 48 + 2 * k:49 + 2 * k], pp, axis=mybir.AxisListType.X
                    )
                    nc.vector.memset(res_sb[:, 49 + 2 * k:50 + 2 * k], 0.0)

            def emit_B(rb, b):
                # DVE fast-exp chunk over columns [b*1024, b*1024+1024).
                # The diagonal (columns rb*128..+128) lies in b=0; mask it
                # with the I.T @ maskr accumulating matmul.
                psB = psumB.tile([128, 1024], f32, name="psB")
                cs0 = rb // 4
                for k in range(KCH):
                    lhsT = lhsT_slice(k, rb)
                    for cs in range(2):
                        nc.tensor.matmul(
                            psB[:, cs * 512:(cs + 1) * 512],
                            lhsT=lhsT,
                            rhs=rhs_slice(k, b * 1024 + cs * 512),
                            start=(k == 0),
                            stop=(k == KCH - 1) and not (b == 0 and cs == cs0),
                        )
                if b == 0:
                    nc.tensor.matmul(
                        psB[:, cs0 * 512:(cs0 + 1) * 512],
                        lhsT=eye_pos,
                        rhs=maskr_sb[:, rb % 4, :],
                        start=False,
                        stop=True,
                    )
                # bits = round(ps * 2A + (B - A*M_r)); sum the bitcast floats.
                ti = ipool.tile([128, 1024], u32, name="ti")
                nc.vector.tensor_scalar(
                    ti, psB, 2.0 * EXP_A, bias2_sb[:, rb:rb + 1],
                    mybir.AluOpType.mult, mybir.AluOpType.add,
                )
                nc.vector.reduce_sum(
                    res_sb[:, rb * 6 + 4 + b:rb * 6 + 5 + b],
                    ti.bitcast(f32),
                    axis=mybir.AxisListType.X,
                )

            def emit_A(rb, a):
                # ACT chunk over columns [2048 + a*1536, +1536).
                psA = psumA.tile([128, 1536], f32, name="psA")
                for k in range(KCH):
                    lhsT = lhsT_slice(k, rb)
                    for cs in range(3):
                        nc.tensor.matmul(
                            psA[:, cs * 512:(cs + 1) * 512],
                            lhsT=lhsT,
                            rhs=rhs_slice(k, 2048 + a * 1536 + cs * 512),
                            start=(k == 0),
                            stop=(k == KCH - 1),
                        )
                nc.scalar.activation(
                    out=psA,
                    in_=psA,
                    func=mybir.ActivationFunctionType.Exp,
                    bias=bias_sb[:, rb:rb + 1],
                    scale=2.0,
                    accum_out=res_sb[:, rb * 6 + a:rb * 6 + a + 1],
                )

            for rb in range(NRB):
                if rb == 5:
                    emit_posdot()
                # B (DVE) chunks interleaved between A (ACT) chunks so the
                # single B PSUM slot never stalls the PE, and each row-block
                # ends on an ACT chunk (short kernel tail).
                if rb == 0:
                    # First row-block consumes columns strictly in DMA
                    # arrival order.
                    for c in (("B", 0), ("B", 1), ("A", 0), ("A", 1),
                              ("A", 2), ("A", 3)):
                        (emit_B if c[0] == "B" else emit_A)(rb, c[1])
                else:
                    emit_B(rb, 0)
                    emit_A(rb, 0)
                    emit_B(rb, 1)
                    emit_A(rb, 1)
                    emit_A(rb, 2)
                    emit_A(rb, 3)

            # Ship rb0-6 partials while rb7 is still computing; only a
            # tiny transfer remains on the kernel tail.
            nc.sync.dma_start(out=out_dram[:, 0:42], in_=res_sb[:, 0:42])
            nc.sync.dma_start(out=out_dram[:, 42:52], in_=res_sb[:, 42:52])

    nc.compile()
    _cache["nc"] = nc
    return nc


def _make_static_inputs(h_i, h_j):
    """Per-core rotated h.T (bf16) plus the diag mask (shared)."""
    h = np.concatenate([np.asarray(h_i), np.asarray(h_j)], axis=0).astype(np.float32)
    hT = np.ascontiguousarray(h.T)  # [256, 8192]
    hts = []
    for c in range(NCORES):
        htc = np.roll(hT, -RPC * c, axis=1)
        hts.append(
            np.ascontiguousarray(htc.astype(ml_dtypes.bfloat16).reshape(KCH, 128, N))
        )
    eye = np.zeros((1, 128, 128), dtype=ml_dtypes.bfloat16)
    p = np.arange(128)
    eye[0, p, p] = 1.0
    maskr = np.zeros((128, 4, 512), dtype=ml_dtypes.bfloat16)
    for v in range(4):
        maskr[p, v, 128 * v + p] = MASK_NEG
    return hts, eye, maskr


def _axon_reset():
    """Recover the axon-tunneled NeuronCores if a previous process left them
    in an unrecoverable state."""
    try:
        import ctypes

        lib = ctypes.CDLL("/opt/axon/libaxon_pjrt.so")
        lib.axon_reset.restype = ctypes.c_int64
        return lib.axon_reset() == 0
    except Exception:
        return False


def _run(nc, hts, eye, maskr, M_per_core):
    global LAST_RESULTS
    from concourse import bass_utils

    in_maps = [
        {
            "ht": hts[c],
            "eye": eye,
            "maskr": maskr,
            "biasm": (-M_per_core[c]).astype(np.float32),
            "bias2": (EXP_B - EXP_A * M_per_core[c]).astype(np.float32),
        }
        for c in range(NCORES)
    ]
    try:
        results = bass_utils.run_bass_kernel_spmd(
            nc, in_maps, core_ids=list(range(NCORES)), trace=TRACE
        )
    except Exception:
        # A wedged accelerator (e.g. NRT_EXEC_UNIT_UNRECOVERABLE from an
        # earlier crashed process) survives process restarts; reset and retry.
        if not _axon_reset():
            raise
        results = bass_utils.run_bass_kernel_spmd(
            nc, in_maps, core_ids=list(range(NCORES)), trace=TRACE
        )
    LAST_RESULTS = results
    return results.results


def kernel(h_i, h_j):
    nc = _build()
    hts, eye, maskr = _make_static_inputs(h_i, h_j)

    # Per-core, per-row logsumexp shift M (as the activation bias -M).
    M = [np.full((128, NRB), M_DEFAULT, dtype=np.float64) for _ in range(NCORES)]

    lse = [np.full((128, NRB), np.nan) for _ in range(NCORES)]
    total_pd = 0.0

    for attempt in range(4):
        res = _run(nc, hts, eye, maskr, M)
        any_bad = False
        for c in range(NCORES):
            out = res[c]["out"].astype(np.float64)
            S = out[:, :48].reshape(128, NRB, 6).sum(axis=2)
            if attempt == 0:
                total_pd += out[:, 48:52].sum()
            good = np.isfinite(S) & (S > 0.0)
            upd = good & ~np.isfinite(lse[c])
            lse[c][upd] = M[c][upd] + np.log(S[upd])
            bad = ~np.isfinite(lse[c])
            if bad.any():
                any_bad = True
                # S == 0 -> M too high for those rows; S inf/nan -> too low.
                over = bad & ~np.isfinite(S)
                under = bad & ~over
                M[c][under] -= 75.0
                M[c][over] += 75.0
        if not any_bad:
            break

    total_lse = sum(l.sum() for l in lse)
    loss = (total_lse - 2.0 * total_pd) / float(N)
    return np.array(loss, dtype=np.float32)


if __name__ == "__main__":
    # Smoke test with random data (not the reference inputs).
    rng = np.random.default_rng(0)
    h_i = rng.standard_normal((B, D), dtype=np.float32)
    h_j = rng.standard_normal((B, D), dtype=np.float32)
    print("loss:", kernel(h_i, h_j))



# revision 5
# speedup vs baseline: 1.1440x; 1.1440x over previous
"""NT-Xent contrastive loss (forward) on 8 TRN2 NeuronCores via Bass/Tile.

Math: with h = concat(h_i, h_j) [N=8192, D=256], sim = (h @ h.T) / 0.5,
loss = mean_r( logsumexp_j(sim[r, j], j != r) - pos_r ), where
pos_r = sim[r, partner(r)] = 2 * h_i[q] . h_j[q].  The loss separates:
loss = (sum_r lse_r - sum_r pos_r) / N, and sum_r pos_r = 4 * sum(h_i * h_j).

Sharding: core c owns rows [1024c, 1024c + 1024).  Each core receives the
full transposed h, column-rotated by its row offset, so one SPMD program
serves all 8 cores.

v2: h is fed as fp8 e4m3 and each 512-col sim chunk is ONE DoubleRow
matmul (K=256 packed as two 128-halves along the tile's middle dim),
~2x the bf16 PE rate.  The DVE fast-exp chunks use a bf16 Schraudolph
(u16 bits), halving the reduce cost; one of the two per-row-block
fast-exp chunks moves its tensor_scalar to the gpsimd engine.  fp8
quantization costs ~1e-3 relative loss error (tolerance 2e-2).
"""

import numpy as np
import ml_dtypes

B = 4096
D = 256
N = 2 * B            # 8192 rows/cols of sim
NCORES = 8
RPC = N // NCORES    # 1024 rows per core
KCH = D // 128       # 2 contraction chunks of 128
NRB = RPC // 128     # 8 row-blocks of 128 per core
M_DEFAULT = 161.0    # logsumexp shift; safe while rowmax(2*h@h.T) in [M-70, M+79]
MASK_NEG = -1.0e9

# Schraudolph fast-exp constants, bf16/u16 variant:
#   bits16 = round(x * 2*A16 + (B16 - A16*M)), bitcast u16 -> bf16.
# B16 is the f32-calibrated bias / 2^16 (phase-averaged, exp-weighted
# relative error ~1e-5; per-term max +-4%).
EXP_A16 = float(2 ** 7 / np.log(2.0))
EXP_B16 = 1064865216.0 / 65536.0

TRACE = False        # set True (e.g. from test.py) to request an NTFF trace
LAST_RESULTS = None  # BassKernelResults of the last run (for profiling)

_cache = {}


def _build():
    """Build the SPMD Bass/Tile program once per process."""
    if "nc" in _cache:
        return _cache["nc"]

    import concourse.tile as tile
    import concourse.mybir as mybir
    from concourse import bacc

    f32 = mybir.dt.float32
    bf16 = mybir.dt.bfloat16
    f8 = mybir.dt.float8e4
    u16 = mybir.dt.uint16

    nc = bacc.Bacc("TRN2", target_bir_lowering=False, num_devices=NCORES)
    ht_dram = nc.dram_tensor("ht", [KCH, 128, N], f8, kind="ExternalInput").ap()
    # eye[0][0] = I [128,128]; mask[v] [128,512] holds -1e9 at [p, 128v+p].
    # I.T @ mask[v] accumulated into a sim-block 512-slice masks its diagonal.
    eye_dram = nc.dram_tensor("eye", [1, 128, 128], bf16, kind="ExternalInput").ap()
    maskr_dram = nc.dram_tensor("maskr", [128, 4, 512], bf16, kind="ExternalInput").ap()
    bias_dram = nc.dram_tensor("biasm", [128, NRB], f32, kind="ExternalInput").ap()
    bias2_dram = nc.dram_tensor("bias2", [128, NRB], f32, kind="ExternalInput").ap()
    out_dram = nc.dram_tensor("out", [128, 52], f32, kind="ExternalOutput").ap()

    DR = mybir.MatmulPerfMode.DoubleRow

    with tile.TileContext(nc) as tc:
        with (
            tc.tile_pool(name="hpool", bufs=1) as hpool,
            tc.tile_pool(name="small", bufs=1) as small,
            tc.tile_pool(name="scratch", bufs=1) as scratch,
            tc.tile_pool(name="ipool", bufs=3) as ipool,
            tc.tile_pool(name="psumA", bufs=2, space="PSUM") as psumA,
            tc.tile_pool(name="psumB", bufs=1, space="PSUM") as psumB,
        ):
            # Small constants go on the gpsimd (SWDGE) queue so they land
            # while the sync queue streams the big h.T chunks.
            eye_pos = small.tile([128, 128], bf16)
            nc.gpsimd.dma_start(out=eye_pos, in_=eye_dram[0])
            maskr_sb = small.tile([128, 4, 512], bf16)
            nc.gpsimd.dma_start(out=maskr_sb, in_=maskr_dram)
            bias_sb = small.tile([128, NRB], f32)
            nc.gpsimd.dma_start(out=bias_sb, in_=bias_dram)
            bias2_sb = small.tile([128, NRB], f32)
            nc.gpsimd.dma_start(out=bias2_sb, in_=bias2_dram)

            # Warm the ACT exp table (~2.7us load) during the DMA prologue so
            # the first real exp doesn't pay for it.
            warm_sb = small.tile([128, 1], f32)
            nc.scalar.activation(
                out=warm_sb, in_=bias_sb[:, 0:1],
                func=mybir.ActivationFunctionType.Exp, bias=0.0, scale=0.0,
            )

            # Bridge the PE-idle window between program start and the first
            # h.T chunk landing so the HAM activity monitor sees sustained
            # work as early as possible (warm clock = 2.4GHz after ~3.4us).
            wsrc = small.tile([128, 128], bf16)
            nc.vector.memset(wsrc, 0.0)
            wps = psumA.tile([128, 1536], f32, name="psA")
            for w in range(12):
                nc.tensor.matmul(
                    wps[:, (w % 3) * 512:(w % 3) * 512 + 128],
                    lhsT=wsrc, rhs=wsrc,
                    start=True, stop=True,
                )

            # h.T in SBUF on the sync HWDGE queue, in the order compute
            # consumes it.  Each DMA carries BOTH contraction halves of a
            # column range (tile layout [128, 2, width]): exactly the
            # DoubleRow access-pattern shape.
            col_ranges = [(0, 1024), (1024, 2048), (2048, 3584),
                          (3584, 5120), (5120, 6656), (6656, 8192)]
            ht_tiles = []
            for di, (c0, c1) in enumerate(col_ranges):
                t = hpool.tile([128, KCH, c1 - c0], f8, name=f"ht_{c0}")
                nc.sync.dma_start(
                    out=t,
                    in_=ht_dram[:, :, c0:c1].rearrange("k p c -> p k c"),
                )
                ht_tiles.append(t)

            def rhs_slice(c0, w=512):
                """[128, 2, w] slice of rotated h.T at global column c0."""
                for (r0, r1), t in zip(col_ranges, ht_tiles):
                    if r0 <= c0 < r1:
                        assert c0 + w <= r1
                        return t[:, :, c0 - r0:c0 - r0 + w]
                raise AssertionError(c0)

            def lhsT_dr(rb):
                """[128, 2, 128] row-block weights (columns rb*128..+128)."""
                return ht_tiles[0][:, :, rb * 128:(rb + 1) * 128]

            res_sb = small.tile([128, 52], f32)

            def emit_posdot():
                # Positive-pair partial dots: rotated columns [0,1024) are
                # this core's rows, [4096,5120) their partners.  Fused
                # multiply+row-accumulate on the gpsimd engine (SBUF-only),
                # keeping the DVE free for the fast-exp chunks.
                for k in range(KCH):
                    pp = scratch.tile([128, RPC], bf16, name=f"ppscratch_{k}")
                    nc.gpsimd.tensor_mul(
                        pp, ht_tiles[0][:, k, :], ht_tiles[3][:, k, 512:512 + RPC]
                    )
                    nc.vector.reduce_sum(
                        res_sb[:, 48 + k:49 + k], pp, axis=mybir.AxisListType.X
                    )
                nc.vector.memset(res_sb[:, 50:52], 0.0)

            def emit_B(rb, b):
                # Fast-exp chunk over columns [b*1024, b*1024+1024).
                # The diagonal (columns rb*128..+128) lies in b=0; mask it
                # with the I.T @ maskr accumulating matmul.
                psB = psumB.tile([128, 1024], f32, name="psB")
                cs0 = rb // 4
                for cs in range(2):
                    nc.tensor.matmul(
                        psB[:, cs * 512:(cs + 1) * 512],
                        lhsT=lhsT_dr(rb),
                        rhs=rhs_slice(b * 1024 + cs * 512),
                        start=True,
                        stop=not (b == 0 and cs == cs0),
                        perf_mode=DR,
                    )
                if b == 0:
                    nc.tensor.matmul(
                        psB[:, cs0 * 512:(cs0 + 1) * 512],
                        lhsT=eye_pos,
                        rhs=maskr_sb[:, rb % 4, :],
                        start=False,
                        stop=True,
                    )
                # bits16 = round(ps * 2*A16 + (B16 - A16*M_r)); sum bf16 bitcast.
                ti = ipool.tile([128, 1024], u16, name="ti")
                nc.vector.tensor_scalar(
                    ti, psB, 2.0 * EXP_A16, bias2_sb[:, rb:rb + 1],
                    mybir.AluOpType.mult, mybir.AluOpType.add,
                )
                nc.vector.reduce_sum(
                    res_sb[:, rb * 6 + 4 + b:rb * 6 + 5 + b],
                    ti.bitcast(bf16),
                    axis=mybir.AxisListType.X,
                )

            def emit_A(rb, a):
                # ACT chunk over columns [2048 + a*1536, +1536).
                psA = psumA.tile([128, 1536], f32, name="psA")
                for cs in range(3):
                    nc.tensor.matmul(
                        psA[:, cs * 512:(cs + 1) * 512],
                        lhsT=lhsT_dr(rb),
                        rhs=rhs_slice(2048 + a * 1536 + cs * 512),
                        start=True,
                        stop=True,
                        perf_mode=DR,
                    )
                nc.scalar.activation(
                    out=psA,
                    in_=psA,
                    func=mybir.ActivationFunctionType.Exp,
                    bias=bias_sb[:, rb:rb + 1],
                    scale=2.0,
                    accum_out=res_sb[:, rb * 6 + a:rb * 6 + a + 1],
                )

            for rb in range(NRB):
                if rb == 5:
                    emit_posdot()
                # B (DVE/GPS) chunks interleaved between A (ACT) chunks so
                # the single B PSUM slot never stalls the PE, and each
                # row-block ends on an ACT chunk (short kernel tail).
                if rb == 0:
                    # First row-block consumes columns strictly in DMA
                    # arrival order.
                    for c in (("B", 0), ("B", 1), ("A", 0), ("A", 1),
                              ("A", 2), ("A", 3)):
                        (emit_B if c[0] == "B" else emit_A)(rb, c[1])
                else:
                    emit_B(rb, 0)
                    emit_A(rb, 0)
                    emit_B(rb, 1)
                    emit_A(rb, 1)
                    emit_A(rb, 2)
                    emit_A(rb, 3)

            # Ship rb0-6 partials while rb7 is still computing; only a
            # tiny transfer remains on the kernel tail.
            nc.sync.dma_start(out=out_dram[:, 0:42], in_=res_sb[:, 0:42])
            nc.sync.dma_start(out=out_dram[:, 42:52], in_=res_sb[:, 42:52])

    nc.compile()
    _cache["nc"] = nc
    return nc


def _make_static_inputs(h_i, h_j):
    """Per-core rotated h.T (fp8 e4m3) plus the diag mask (shared)."""
    h = np.concatenate([np.asarray(h_i), np.asarray(h_j)], axis=0).astype(np.float32)
    hT = np.ascontiguousarray(h.T)  # [256, 8192]
    hts = []
    for c in range(NCORES):
        htc = np.roll(hT, -RPC * c, axis=1)
        hts.append(
            np.ascontiguousarray(htc.astype(ml_dtypes.float8_e4m3).reshape(KCH, 128, N))
        )
    eye = np.zeros((1, 128, 128), dtype=ml_dtypes.bfloat16)
    p = np.arange(128)
    eye[0, p, p] = 1.0
    maskr = np.zeros((128, 4, 512), dtype=ml_dtypes.bfloat16)
    for v in range(4):
        maskr[p, v, 128 * v + p] = MASK_NEG
    return hts, eye, maskr


def _axon_reset():
    """Recover the axon-tunneled NeuronCores if a previous process left them
    in an unrecoverable state."""
    try:
        import ctypes

        lib = ctypes.CDLL("/opt/axon/libaxon_pjrt.so")
        lib.axon_reset.restype = ctypes.c_int64
        return lib.axon_reset() == 0
    except Exception:
        return False


def _run(nc, hts, eye, maskr, M_per_core):
    global LAST_RESULTS
    from concourse import bass_utils

    in_maps = [
        {
            "ht": hts[c],
            "eye": eye,
            "maskr": maskr,
            "biasm": (-M_per_core[c]).astype(np.float32),
            "bias2": (EXP_B16 - EXP_A16 * M_per_core[c]).astype(np.float32),
        }
        for c in range(NCORES)
    ]
    try:
        results = bass_utils.run_bass_kernel_spmd(
            nc, in_maps, core_ids=list(range(NCORES)), trace=TRACE
        )
    except Exception:
        # A wedged accelerator (e.g. NRT_EXEC_UNIT_UNRECOVERABLE from an
        # earlier crashed process) survives process restarts; reset and retry.
        if not _axon_reset():
            raise
        results = bass_utils.run_bass_kernel_spmd(
            nc, in_maps, core_ids=list(range(NCORES)), trace=TRACE
        )
    LAST_RESULTS = results
    return results.results


def kernel(h_i, h_j):
    nc = _build()
    hts, eye, maskr = _make_static_inputs(h_i, h_j)

    # Per-core, per-row logsumexp shift M (as the activation bias -M).
    M = [np.full((128, NRB), M_DEFAULT, dtype=np.float64) for _ in range(NCORES)]

    lse = [np.full((128, NRB), np.nan) for _ in range(NCORES)]
    total_pd = 0.0

    for attempt in range(4):
        res = _run(nc, hts, eye, maskr, M)
        any_bad = False
        for c in range(NCORES):
            out = res[c]["out"].astype(np.float64)
            S = out[:, :48].reshape(128, NRB, 6).sum(axis=2)
            if attempt == 0:
                total_pd += out[:, 48:52].sum()
            good = np.isfinite(S) & (S > 0.0)
            upd = good & ~np.isfinite(lse[c])
            lse[c][upd] = M[c][upd] + np.log(S[upd])
            bad = ~np.isfinite(lse[c])
            if bad.any():
                any_bad = True
                # S == 0 -> M too high for those rows; S inf/nan -> too low.
                over = bad & ~np.isfinite(S)
                under = bad & ~over
                M[c][under] -= 75.0
                M[c][over] += 75.0
        if not any_bad:
            break

    total_lse = sum(l.sum() for l in lse)
    loss = (total_lse - 2.0 * total_pd) / float(N)
    return np.array(loss, dtype=np.float32)


if __name__ == "__main__":
    # Smoke test with random data (not the reference inputs).
    rng = np.random.default_rng(0)
    h_i = rng.standard_normal((B, D), dtype=np.float32)
    h_j = rng.standard_normal((B, D), dtype=np.float32)
    print("loss:", kernel(h_i, h_j))
